# revision 1
# baseline (speedup 1.0000x reference)
"""AGCRN cell (edge-featured GCN GRU step from H=0) on 8 TRN2 NeuronCores.

Math (reference has H = 0, so Z and the R*H term vanish):
  out[t] = (1 - R[t]) * tanh(U[t])
  R[t] = sigmoid(P_t @ WgR + Q @ WeR + bR)      (R-half of the gate conv)
  U[t] = P_t @ Wu + Q @ WeU + bU                (update conv)
  P_t[n] = sum_{e: dst_e = n} norm_e * x_t[src_e]        [N, 64]
  Q[n]   = sum_{e: dst_e = n} norm_e * ea_e              [N, 16]
  norm_e = rsqrt(deg[src_e] * deg[dst_e]),  deg = max(indeg, 1)

Linearity of the GCN moves the matmuls AFTER the scatter, so all
gather/scatter traffic is in 64+16 dims rather than 256.

Distribution: one core per timestep t (data-parallel over T=8; graph and
weights replicated, no collectives).

Per-core device pipeline:
  1. dma_gather x[src_e] (256B rows) -> SBUF, scale by norm on DVE
  2. dma_scatter_add into P scratch HBM rows (CCE add); edges are
     pre-grouped by destination-occurrence so each scatter call has
     unique dst indices (no RMW races); Tile serializes the calls (WAW).
  3. read P back, TensorE-transpose blocks, matmul with combined [81,256]
     weight (row 80 = bias via a ones-column in P), sigmoid/tanh epilogue.
"""

import ml_dtypes
import numpy as np

import concourse.bacc as bacc
import concourse.mybir as mybir
import concourse.tile as tile
from concourse.bass_utils import run_bass_kernel_spmd
from concourse.masks import make_identity

F32 = mybir.dt.float32
BF16 = mybir.dt.bfloat16
I16 = mybir.dt.int16

P = 128


class Cfg:
    def __init__(self, n_nodes, f_in, f_edge, f_out, chunks):
        self.n = n_nodes
        self.f_in = f_in
        self.f_edge = f_edge
        self.f_out = f_out
        self.chunks = list(chunks)  # (n_128blocks, real_tokens, kind) per call
        self.group_nb = [nb for nb, _, _ in self.chunks]
        self.nblk = (n_nodes + P - 1) // P
        self.n_pad = self.nblk * P
        self.c = f_in + f_edge + 1  # contraction: x feats + ea feats + ones(bias)
        self.pcols = 128            # P-scratch row width (512B stride)
        self.gblk = sum(self.group_nb)
        self.ecol16 = self.gblk * 8  # cols of the [*,16]-wrapped index tensors
        self.nbmax = max(self.group_nb)
        self.fo2 = 2 * f_out


def build(nc, cfg):
    import os as _os
    _skip = set(_os.environ.get("K_SKIP", "").split(","))
    _ab = _os.environ.get("K_AB", "0") == "1"
    _bufs = int(_os.environ.get("K_BUFS", "3"))
    mult = mybir.AluOpType.mult
    subtract = mybir.AluOpType.subtract
    Sigmoid = mybir.ActivationFunctionType.Sigmoid
    Tanh = mybir.ActivationFunctionType.Tanh

    x = nc.declare_dram_parameter("x", [cfg.n, cfg.f_in], F32, isOutput=False)
    eaw = nc.declare_dram_parameter("eaw", [P, cfg.gblk, cfg.f_edge], BF16, isOutput=False)
    normw = nc.declare_dram_parameter("normw", [P, cfg.gblk], F32, isOutput=False)
    gidx = nc.declare_dram_parameter("gidx", [P, cfg.ecol16], I16, isOutput=False)
    sidx = nc.declare_dram_parameter("sidx", [P, cfg.ecol16], I16, isOutput=False)
    wcomb = nc.declare_dram_parameter("wcomb", [cfg.c, cfg.fo2], BF16, isOutput=False)
    out = nc.declare_dram_parameter("out", [cfg.n, cfg.f_out], F32, isOutput=True)
    # zero-initialized scratch for the scatter accumulation (outputs arrive
    # zeroed from the runtime; its returned value is ignored by the host)
    pbuf = nc.declare_dram_parameter(
        "pscratch", [cfg.n_pad, cfg.pcols], BF16, isOutput=True
    )
    pbufb = nc.declare_dram_parameter(
        "pscratch2", [cfg.n_pad, cfg.pcols], BF16, isOutput=True
    )
    pview = pbuf[:].rearrange("(b p) f -> p b f", p=P)
    fe = cfg.f_in + cfg.f_edge  # merged scatter row width (80)

    with tile.TileContext(nc) as tc:
        with (
            tc.tile_pool(name="gat", bufs=_bufs) as gat_pool,
            tc.tile_pool(name="mg", bufs=_bufs) as mg_pool,
            tc.tile_pool(name="misc", bufs=1) as misc_pool,
            tc.tile_pool(name="ep", bufs=2) as ep_pool,
            tc.tile_pool(name="mpsum", bufs=2, space="PSUM") as mpsum_pool,
        ):
            # --- resident tiles ---
            norm_sb = misc_pool.tile([P, cfg.gblk], F32)
            nc.sync.dma_start(out=norm_sb[:], in_=normw[:])
            normb_sb = misc_pool.tile([P, cfg.gblk], BF16)
            nc.vector.tensor_copy(out=normb_sb[:], in_=norm_sb[:])
            h = (cfg.ecol16 // 2) & ~7
            gidx_sb = misc_pool.tile([P, cfg.ecol16], I16)
            nc.sync.dma_start(out=gidx_sb[:, :h], in_=gidx[:, :h])
            sidx_sb = misc_pool.tile([P, cfg.ecol16], I16)
            nc.sync.dma_start(out=sidx_sb[:, :h], in_=sidx[:, :h])
            nc.sync.dma_start(out=gidx_sb[:, h:], in_=gidx[:, h:])
            nc.sync.dma_start(out=sidx_sb[:, h:], in_=sidx[:, h:])
            ea_sb = misc_pool.tile([P, cfg.gblk, cfg.f_edge], BF16)
            nc.sync.dma_start(out=ea_sb[:], in_=eaw[:])
            # pre-scale edge attrs by norm once (t-independent), sliced so
            # early chunks unblock before the whole pass finishes
            q4 = max(1, cfg.gblk // 8)
            for _q0 in range(0, cfg.gblk, q4):
                _q1 = min(_q0 + q4, cfg.gblk)
                nc.vector.tensor_tensor(
                    out=ea_sb[:, _q0:_q1, :],
                    in0=ea_sb[:, _q0:_q1, :],
                    in1=normb_sb[:, _q0:_q1].unsqueeze(2).to_broadcast(
                        [P, _q1 - _q0, cfg.f_edge]
                    ),
                    op=mult,
                )
            w_sb = misc_pool.tile([P, cfg.fo2], BF16)
            nc.sync.dma_start(out=w_sb[: cfg.c, :], in_=wcomb[:])
            ones_sb = misc_pool.tile([P, cfg.nblk], BF16)
            nc.gpsimd.memset(ones_sb[:], 1.0)
            # ones column (col fe) of pbuf -> bias row of wcomb
            nc.sync.dma_start(
                out=pview[:, :, fe : fe + 1],
                in_=ones_sb[:].unsqueeze(2),
            )

            # --- gather / scale / merged scatter, one call per chunk ---
            col = 0
            col0 = 0  # node-block offset within the full-range write chunks
            nadd = 0
            for ci, (nb, reg, kind) in enumerate(() if "scat" in _skip else cfg.chunks):
                cnt = nb * P
                gat = gat_pool.tile([P, cfg.nbmax, cfg.f_in], F32, tag="gat")
                gsl = gat[:, :nb, :]
                if reg < cnt:
                    # gather skips the trailing -1 pad slots; pre-zero the last
                    # block column so the norm-scale below reads defined data
                    nc.vector.memset(gat[:, nb - 1 : nb, :], 0.0)
                if "gat" in _skip:
                    nc.vector.memset(gsl, 0.0)
                else:
                    nc.gpsimd.dma_gather(
                    out_ap=gsl,
                    in_ap=x[:],
                    idxs_ap=gidx_sb[:, col * 8 : col * 8 + cnt // 16],
                    num_idxs=cnt,
                    num_idxs_reg=reg,
                    elem_size=cfg.f_in,
                    single_packet=False,
                    )
                easl = ea_sb[:, col : col + nb, :]

                mg = mg_pool.tile([P, cfg.nbmax, fe], BF16, tag="mg")
                nsl = norm_sb[:, col : col + nb].unsqueeze(2)
                nc.vector.tensor_tensor(
                    out=mg[:, :nb, 0 : cfg.f_in],
                    in0=gsl,
                    in1=nsl.to_broadcast([P, nb, cfg.f_in]),
                    op=mult,
                )
                nc.vector.tensor_copy(
                    out=mg[:, :nb, cfg.f_in : fe], in_=easl
                )
                if "sct" in _skip:
                    pass
                elif kind == "w":
                    # full-range dst-ordered chunk: plain write, no indices
                    nc.sync.dma_start(
                        out=pview[:, col0:col0 + nb, 0:fe],
                        in_=mg[:, :nb, :],
                    )
                else:
                    nadd += 1
                    nc.gpsimd.dma_scatter_add(
                    out_ap=(pbuf if (nadd % 2 or not _ab) else pbufb)[:, 0:fe],
                    in_ap=mg[:, :nb, :],
                    idxs_ap=sidx_sb[:, col * 8 : col * 8 + cnt // 16],
                    num_idxs=cnt,
                    num_idxs_reg=reg,
                    elem_size=fe,
                    elem_step=cfg.pcols,
                    single_packet=False,
                    )
                col += nb
                if kind == "w":
                    col0 += nb

            # --- P^T via HWDGE transpose-DMA (P scratch is already bf16) ---
            pt_sb = misc_pool.tile([P, cfg.n_pad], BF16)
            pt_sb2 = misc_pool.tile([P, cfg.n_pad], BF16) if _ab else None
            if "tr" in _skip:
                nc.vector.memset(pt_sb[:], 0.0)
            else:
                qn = (cfg.n_pad // 4) & ~127
                _cuts = [0, qn, 2 * qn, 3 * qn, cfg.n_pad]
                for _a, _b in zip(_cuts, _cuts[1:]):
                    nc.sync.dma_start(
                        out=pt_sb[:, _a:_b], in_=pbuf[_a:_b, :], transpose=True
                    )
                if _ab:
                    nc.sync.dma_start(
                        out=pt_sb2[:], in_=pbufb[:], transpose=True
                    )

            if "mm" in _skip:
                pass
            GS = 8
            for g0 in (() if "mm" in _skip else range(0, cfg.nblk, GS)):
                gs = min(GS, cfg.nblk - g0)
                mpsum = mpsum_pool.tile([P, GS * cfg.fo2], F32, tag="mpsum")
                for j in range(gs):
                    b = g0 + j
                    nc.tensor.matmul(
                        out=mpsum[:, j * cfg.fo2 : (j + 1) * cfg.fo2],
                        lhsT=pt_sb[: cfg.c, b * P : (b + 1) * P],
                        rhs=w_sb[: cfg.c, :],
                        start=True,
                        stop=not _ab,
                    )
                    if _ab:
                        nc.tensor.matmul(
                            out=mpsum[:, j * cfg.fo2 : (j + 1) * cfg.fo2],
                            lhsT=pt_sb2[: cfg.c, b * P : (b + 1) * P],
                            rhs=w_sb[: cfg.c, :],
                            start=False,
                            stop=True,
                        )
                mview = mpsum[:].rearrange("p (g f) -> p g f", f=cfg.fo2)
                sg = ep_pool.tile([P, GS, cfg.f_out], BF16, tag="sg")
                th = ep_pool.tile([P, GS, cfg.f_out], BF16, tag="th")
                nc.scalar.activation(
                    out=sg[:, :gs, :],
                    in_=mview[:, :gs, 0 : cfg.f_out],
                    func=Sigmoid,
                    scale=-1.0,
                )
                nc.scalar.activation(
                    out=th[:, :gs, :],
                    in_=mview[:, :gs, cfg.f_out : cfg.fo2],
                    func=Tanh,
                )
                outg = ep_pool.tile([P, GS, cfg.f_out], F32, tag="outg")
                nc.vector.tensor_tensor(
                    out=outg[:, :gs, :],
                    in0=th[:, :gs, :],
                    in1=sg[:, :gs, :],
                    op=mult,
                )
                # stream this group's rows out (overlaps later groups)
                nfull = cfg.n // P
                ntail = cfg.n - nfull * P
                ov = out[0 : nfull * P, :].rearrange("(b p) f -> p b f", p=P)
                hi = min(g0 + gs, nfull)
                if hi > g0:
                    nc.sync.dma_start(
                        out=ov[:, g0:hi, :], in_=outg[:, : hi - g0, :]
                    )
                if ntail and g0 + gs > nfull:
                    nc.sync.dma_start(
                        out=out[nfull * P : cfg.n, :],
                        in_=outg[:ntail, nfull - g0, :],
                    )
    return nc


def _wrap16(v):
    """token i -> [i % 16, i // 16], replicated to 128 partitions (int16)."""
    a = np.ascontiguousarray(np.asarray(v, np.int16).reshape(-1, 16).T)
    return np.ascontiguousarray(np.tile(a, (8, 1)))


def _wrap128(v):
    """token i -> [i % 128, i // 128] (leading dim 128)."""
    a = np.asarray(v)
    return np.ascontiguousarray(a.reshape(-1, P, *a.shape[1:]).swapaxes(0, 1))


def host_prep(x_tn, edge_index, edge_attr, Wn_gate, We_gate, b_gate, Wn_upd,
              We_upd, b_upd):
    """Index preprocessing + weight packing. Returns (cfg, shared in_map)."""
    n = x_tn.shape[1]
    f_in = x_tn.shape[2]
    ea = np.asarray(edge_attr, np.float32)
    f_edge = ea.shape[1]
    f_out = np.asarray(Wn_upd).shape[1]

    src = np.asarray(edge_index[0], np.int64)
    dst = np.asarray(edge_index[1], np.int64)
    e = src.shape[0]

    deg = np.bincount(dst, minlength=n).astype(np.float32)
    deg = np.maximum(deg, 1.0)
    norm = (1.0 / np.sqrt(deg[src] * deg[dst])).astype(np.float32)

    # group edges by occurrence-rank within their dst -> unique dst per group
    order = np.argsort(dst, kind="stable")
    sdst = dst[order]
    starts = np.searchsorted(sdst, np.arange(n))
    occ = np.arange(e) - starts[sdst]  # occurrence rank in sorted order
    max_occ = int(occ.max())

    import os as _os
    MAXBLK = int(_os.environ.get("K_MAXBLK", "48"))  # tokens/call = MAXBLK*128
    gsrc, gdst, gnorm, gea, chunks = [], [], [], [], []

    # group 0: the first edge of every node, in node order, padded to the
    # full [0, n_pad) range -> its "scatter" is a plain sequential write.
    n_pad = ((n + P - 1) // P) * P
    first_ids = np.full(n_pad, -1, np.int64)
    ids0 = order[occ == 0]
    first_ids[dst[ids0]] = ids0
    g0_src = np.zeros(n_pad, np.int64)
    g0_norm = np.zeros(n_pad, np.float32)
    g0_ea = np.zeros((n_pad, f_edge), np.float32)
    have = first_ids >= 0
    g0_src[have] = src[first_ids[have]]
    g0_norm[have] = norm[first_ids[have]]
    g0_ea[have] = ea[first_ids[have]]
    # deg-0 nodes keep src=0/norm=0 (defined gather, zero write);
    # rows >= n are -1 (gather-skipped, memset path)
    g0_gidx = g0_src.copy()
    g0_gidx[n:] = -1
    g0_dst = np.arange(n_pad, dtype=np.int64)
    g0_dst[n:] = -1
    off = 0
    while off < n_pad:
        hi = min(off + MAXBLK * P, n_pad)
        chunks.append(((hi - off) // P, max(0, min(n, hi) - off), "a"))
        gsrc.append(g0_gidx[off:hi])
        gdst.append(g0_dst[off:hi])
        gnorm.append(g0_norm[off:hi])
        gea.append(g0_ea[off:hi])
        off = hi

    for k in range(1, max_occ + 1):
        ids = order[occ == k]
        off = 0
        while off < ids.shape[0]:
            sub = ids[off : off + MAXBLK * P]
            m = sub.shape[0]
            pad = (-m) % P
            chunks.append(((m + pad) // P, m, "a"))
            gsrc.append(np.concatenate([src[sub], np.full(pad, -1, np.int64)]))
            gdst.append(np.concatenate([dst[sub], np.full(pad, -1, np.int64)]))
            gnorm.append(np.concatenate([norm[sub], np.zeros(pad, np.float32)]))
            gea.append(
                np.concatenate(
                    [ea[sub], np.zeros((pad, f_edge), np.float32)], axis=0
                )
            )
            off += m
    # small chunks first: primes the gather/scale/scatter pipeline
    order_c = sorted(range(len(chunks)), key=lambda i: -chunks[i][0])
    chunks = [chunks[i] for i in order_c]
    gsrc = [gsrc[i] for i in order_c]
    gdst = [gdst[i] for i in order_c]
    gnorm = [gnorm[i] for i in order_c]
    gea = [gea[i] for i in order_c]
    src_all = np.concatenate(gsrc)
    dst_all = np.concatenate(gdst)
    norm_all = np.concatenate(gnorm).astype(np.float32)
    ea_all = np.concatenate(gea, axis=0).astype(np.float32)

    cfg = Cfg(n, f_in, f_edge, f_out, chunks)

    wc = np.zeros((cfg.c, cfg.fo2), np.float32)
    Wn_gate = np.asarray(Wn_gate, np.float32)
    We_gate = np.asarray(We_gate, np.float32)
    b_gate = np.asarray(b_gate, np.float32)
    wc[0:f_in, 0:f_out] = Wn_gate[:f_in, f_out : 2 * f_out]
    wc[0:f_in, f_out:] = np.asarray(Wn_upd, np.float32)[:f_in, :]
    wc[f_in : f_in + f_edge, 0:f_out] = We_gate[:, f_out : 2 * f_out]
    wc[f_in : f_in + f_edge, f_out:] = np.asarray(We_upd, np.float32)
    wc[cfg.c - 1, 0:f_out] = b_gate[f_out : 2 * f_out]
    wc[cfg.c - 1, f_out:] = np.asarray(b_upd, np.float32)

    shared = {
        "eaw": _wrap128(ea_all).astype(ml_dtypes.bfloat16),
        "normw": _wrap128(norm_all),
        "gidx": _wrap16(src_all),
        "sidx": _wrap16(dst_all),
        "wcomb": wc.astype(ml_dtypes.bfloat16),
    }
    return cfg, shared


def pack_x(x_nt):
    return np.ascontiguousarray(np.asarray(x_nt, np.float32))


def run(inputs, trace=False, **spmd_kwargs):
    x_tn = np.asarray(inputs["X"], np.float32)[0]  # [T, N, F_IN]
    t_steps = x_tn.shape[0]
    cfg, shared = host_prep(
        x_tn,
        inputs["edge_index"],
        inputs["edge_attr"],
        inputs["Wn_gate"],
        inputs["We_gate"],
        inputs["b_gate"],
        inputs["Wn_upd"],
        inputs["We_upd"],
        inputs["b_upd"],
    )
    import os as _os
    nc = bacc.Bacc(
        None, debug=False,
        dynamic_dma_scratch_size=int(_os.environ.get("K_SCRATCH", "16384")),
    )
    build(nc, cfg)
    nc.compile()
    in_maps = [
        {**shared, "x": pack_x(x_tn[t])} for t in range(t_steps)
    ]
    bkr = run_bass_kernel_spmd(
        nc, in_maps, list(range(t_steps)), trace=trace, **spmd_kwargs
    )
    out = np.stack([bkr.results[t]["out"] for t in range(t_steps)], axis=0)
    return out, bkr


def kernel(**inputs):
    return run(inputs)[0]

